# revision 29
# baseline (speedup 1.0000x reference)
"""Edge-parallel multi-head graph attention on 8 Trainium2 NeuronCores.

Strategy (matches the edge-parallel sharding hint):
  - Host: LPT-balance the 20000 destination nodes into 160 blocks of 128
    (8 cores x 20 blocks) so every block has ~4000 incoming edges; permute
    node ids so block b owns padded node ids [128b, 128b+128).  Edges are
    grouped by destination block and padded to G*1024 per block.  The
    one-hot dst matrices (pure functions of edge_index) are precomputed
    here in both layouts and streamed to the device as bf16 inputs,
    keeping them off the (bottleneck) DVE engine.
  - Device (SPMD, one NEFF on 8 cores; all per-core variation is input
    data): each core projects the full K/V tables (K|V interleaved per
    node, [20480, 256] bf16) into its private DRAM, projects Q for its
    own 2560 nodes (pre-scaled by 1/sqrt(d)), then streams blocks:
    1024-index dma_gathers of K|V rows by source id (SWDGE descriptor
    generation is the kernel's bottleneck; all index tiles are preloaded
    and the gathers round-robin 4 SWDGE queues), per-edge Q expansion on
    PE (one-hot lhsT, software-pipelined one group ahead), a ScalarE cast
    of the expansion to packed bf16 so the DVE score multiply runs in its
    2x packed mode, per-head score reduce + exp, V weighting, and PE
    matmuls accumulating the weighted-V and softmax-denominator segments
    in PSUM per block.  Softmax max-shift is skipped (scores are O(5) so
    exp cannot overflow and softmax is shift-invariant) and the eps term
    is dropped, which lets the V-bias fold through Wo into the output
    bias on the host.  Output is written feature-major and unpermuted on
    the host.
"""

import numpy as np

N = 20000
E = 640000
HID = 128
H = 8
D = 16
SCALE = D ** -0.5
EPS = 1e-8

NCORES = 8
P = 128                 # partitions / nodes per block
BPC = 20                # blocks per core
NB = NCORES * BPC       # 160 blocks
NP_PAD = NB * P         # 20480 padded nodes
NSH = BPC * P           # 2560 nodes per core shard
GRP = 1024              # edges per compute group / gather
GSUB = GRP // P         # 128-edge sub-tiles per compute group

_COMPILED = {}          # G -> nc


# ----------------------------------------------------------------- host plan
def _build_plan(edge_index):
    import heapq

    src = np.asarray(edge_index[0]).astype(np.int64)
    dst = np.asarray(edge_index[1]).astype(np.int64)

    deg = np.bincount(dst, minlength=N)
    order = np.argsort(-deg, kind="stable")
    block_fill = np.zeros(NB, dtype=np.int64)
    node_block = np.empty(N, dtype=np.int64)
    node_slot = np.empty(N, dtype=np.int64)
    heap = [(0, b) for b in range(NB)]
    heapq.heapify(heap)
    loads = np.zeros(NB, dtype=np.int64)
    for n in order:
        while True:
            load, b = heapq.heappop(heap)
            if block_fill[b] < P:
                break
        node_block[n] = b
        node_slot[n] = block_fill[b]
        block_fill[b] += 1
        loads[b] = load + deg[n]
        if block_fill[b] < P:
            heapq.heappush(heap, (loads[b], b))
    perm = node_block * P + node_slot          # old node id -> padded id
    G = max(1, int(np.ceil(loads.max() / GRP)))
    cap = G * GRP

    new_dst = perm[dst]
    new_src = perm[src]
    blk = new_dst // P
    order_e = np.argsort(blk, kind="stable")
    es = new_src[order_e]
    ed = new_dst[order_e]
    eb = blk[order_e]

    src_pad = np.zeros((NB, cap), dtype=np.int32)
    dstloc_pad = np.full((NB, cap), -1.0, dtype=np.float32)
    starts = np.searchsorted(eb, np.arange(NB))
    ends = np.searchsorted(eb, np.arange(NB) + 1)
    for b in range(NB):
        s, e = starts[b], ends[b]
        src_pad[b, : e - s] = es[s:e]
        dstloc_pad[b, : e - s] = (ed[s:e] - b * P).astype(np.float32)
    return perm, G, src_pad, dstloc_pad


def _host_inputs(inputs, perm, G, src_pad, dstloc_pad):
    """Build the per-core input maps."""
    q = np.ascontiguousarray(np.asarray(inputs["query"], np.float32))
    k = np.ascontiguousarray(np.asarray(inputs["key"], np.float32))
    v = np.ascontiguousarray(np.asarray(inputs["value"], np.float32))
    Wq = np.asarray(inputs["Wq"], np.float32); bq = np.asarray(inputs["bq"], np.float32)
    Wk = np.asarray(inputs["Wk"], np.float32)
    Wv = np.asarray(inputs["Wv"], np.float32); bv = np.asarray(inputs["bv"], np.float32)
    Wo = np.asarray(inputs["Wo"], np.float32); bo = np.asarray(inputs["bo"], np.float32)

    import ml_dtypes

    bf = ml_dtypes.bfloat16
    cap = G * GRP
    qp = np.zeros((NP_PAD, HID), np.float32); qp[perm] = q
    kp = np.zeros((NP_PAD, HID), np.float32); kp[perm] = k
    vp = np.zeros((NP_PAD, HID), np.float32); vp[perm] = v
    kT = np.ascontiguousarray(kp.T.astype(bf))     # [128, 20480]
    vT = np.ascontiguousarray(vp.T.astype(bf))

    # NOTE: the K projection bias bk shifts every score of a softmax segment
    # by the same amount (it only depends on (dst, head)), so it cancels in
    # softmax and is dropped.  With eps dropped the attention weights of a
    # segment sum to exactly 1, so the V bias folds through Wo into the
    # output bias: bo_eff = bo + Wo @ bv.
    bo_eff = bo + Wo @ bv
    shared = {
        "kT": kT,
        "vT": vT,
        "wq_lhsT": np.ascontiguousarray((Wq * SCALE).T.astype(bf)),
        "wk_lhsT": np.ascontiguousarray(Wk.T.astype(bf)),
        "wv_lhsT": np.ascontiguousarray(Wv.T.astype(bf)),
        "wo_lhsT": np.ascontiguousarray(Wo.T.astype(bf)),
        "bq_row": np.ascontiguousarray((bq * SCALE).reshape(1, HID)),
        "bo_col": np.ascontiguousarray(bo_eff.reshape(HID, 1).astype(np.float32)),
    }

    SRCW = cap // 16                        # src-wrap cols per block
    NSUB = cap // P
    iota = np.arange(P, dtype=np.int16)
    in_maps = []
    for c in range(NCORES):
        blocks = range(c * BPC, (c + 1) * BPC)
        gidx = np.empty((BPC, P, SRCW), np.int16)
        selE = np.empty((BPC, P, cap), ml_dtypes.bfloat16)
        selN = np.empty((BPC, P, cap), ml_dtypes.bfloat16)
        for bi, b in enumerate(blocks):
            wrapped = src_pad[b].reshape(SRCW, 16).T.astype(np.int16)   # [16, SRCW]
            gidx[bi] = np.tile(wrapped, (8, 1))
            flat_d = dstloc_pad[b].astype(np.int16)
            # sel_en[p, jj*128+d] = (dst of edge jj*128+p == d)
            dst_w = flat_d.reshape(NSUB, P).T                            # [p, jj]
            selE[bi] = (dst_w[:, :, None] == iota[None, None, :]).reshape(P, cap)
            # sel_ne[s, e] = (dst of edge e == s)
            selN[bi] = flat_d[None, :] == iota[:, None]
        qT_c = np.ascontiguousarray(qp[c * NSH : (c + 1) * NSH].T.astype(bf))
        m = dict(shared)
        m["qT"] = qT_c
        m["gidx"] = gidx
        m["selE"] = selE
        m["selN"] = selN
        in_maps.append(m)
    return in_maps


# ------------------------------------------------------------- device kernel
def _build_nc(G):
    from contextlib import ExitStack

    import concourse.bacc as bacc
    import concourse.bass as bass
    import concourse.mybir as mybir
    import concourse.tile as tile
    from concourse.masks import make_identity

    f32 = mybir.dt.float32
    bf16 = mybir.dt.bfloat16
    i16 = mybir.dt.int16
    AF = mybir.ActivationFunctionType
    cap = G * GRP
    SRCW = cap // 16
    NSUB = G * GSUB                 # 128-edge sub-tiles per block
    assert G % 2 == 0, "compute groups are 2048 edges (two gathers)"
    G2 = G // 2                     # 2048-edge compute groups per block
    GRP2 = 2 * GRP
    GSUB2 = 2 * GSUB

    nc = bacc.Bacc(
        "TRN2",
        target_bir_lowering=False,
        debug=False,
        num_devices=NCORES,
        dynamic_dma_scratch_size=32768,
        num_swdge_queues=4,
    )

    qT = nc.dram_tensor("qT", [P, NSH], bf16, kind="ExternalInput").ap()
    kT = nc.dram_tensor("kT", [P, NP_PAD], bf16, kind="ExternalInput").ap()
    vT = nc.dram_tensor("vT", [P, NP_PAD], bf16, kind="ExternalInput").ap()
    wq = nc.dram_tensor("wq_lhsT", [P, P], bf16, kind="ExternalInput").ap()
    wk = nc.dram_tensor("wk_lhsT", [P, P], bf16, kind="ExternalInput").ap()
    wv = nc.dram_tensor("wv_lhsT", [P, P], bf16, kind="ExternalInput").ap()
    wo = nc.dram_tensor("wo_lhsT", [P, P], bf16, kind="ExternalInput").ap()
    bqr = nc.dram_tensor("bq_row", [1, P], f32, kind="ExternalInput").ap()
    boc = nc.dram_tensor("bo_col", [P, 1], f32, kind="ExternalInput").ap()
    gidx = nc.dram_tensor("gidx", [BPC, P, SRCW], i16, kind="ExternalInput").ap()
    selE = nc.dram_tensor("selE", [BPC, P, cap], bf16, kind="ExternalInput").ap()
    selN = nc.dram_tensor("selN", [BPC, P, cap], bf16, kind="ExternalInput").ap()
    outT = nc.dram_tensor("outT", [P, NSH], f32, kind="ExternalOutput").ap()

    def ap3(t_ap, dims, extra_offset=0):
        return bass.AP(t_ap.tensor, t_ap.offset + extra_offset, dims)

    with tile.TileContext(nc) as tc, ExitStack() as ctx:
        const = ctx.enter_context(tc.tile_pool(name="const", bufs=1))
        dram = ctx.enter_context(tc.tile_pool(name="dram", bufs=1, space="DRAM"))
        pp = ctx.enter_context(tc.tile_pool(name="pp", bufs=6))
        blkp = ctx.enter_context(tc.tile_pool(name="blkp", bufs=BPC))
        kvp = ctx.enter_context(tc.tile_pool(name="kvp", bufs=6))
        selp = ctx.enter_context(tc.tile_pool(name="selp", bufs=3))
        ep = ctx.enter_context(tc.tile_pool(name="ep", bufs=3))
        psQ = ctx.enter_context(tc.tile_pool(name="psQ", bufs=1, space="PSUM"))
        psT = ctx.enter_context(tc.tile_pool(name="psT", bufs=2, space="PSUM"))
        psO = ctx.enter_context(tc.tile_pool(name="psO", bufs=2, space="PSUM"))

        kvf = dram.tile([NP_PAD, 2 * HID], bf16)

        c_wq = const.tile([P, P], bf16); nc.sync.dma_start(c_wq[:], wq)
        c_wk = const.tile([P, P], bf16); nc.sync.dma_start(c_wk[:], wk)
        c_wv = const.tile([P, P], bf16); nc.sync.dma_start(c_wv[:], wv)
        c_wo = const.tile([P, P], bf16); nc.sync.dma_start(c_wo[:], wo)
        c_bo = const.tile([P, 1], f32); nc.sync.dma_start(c_bo[:], boc)
        c_bqr = const.tile([P, P], f32)
        nc.sync.dma_start(c_bqr[:], ap3(bqr, [[0, P], [1, P]]))
        identb = const.tile([P, P], bf16)
        make_identity(nc, identb[:])
        qsb = const.tile([P, BPC, P], bf16)

        ts = bass.ts

        # ------- phase P: K|V projections interleaved per 512-node tile.
        # out[n, j] via lhsT=x_chunk, rhs=W; K and V land in one PSUM tile
        # so each tile needs a single PSUM->SBUF cast and one DMA write of
        # contiguous 512B node rows.
        W = 512
        JW = W // P
        for t in range(NP_PAD // W):
            xk = pp.tile([P, W], bf16, tag="xk")
            nc.sync.dma_start(xk[:], kT[:, ts(t, W)])
            xv = pp.tile([P, W], bf16, tag="xv")
            nc.sync.dma_start(xv[:], vT[:, ts(t, W)])
            kvh = pp.tile([P, JW, 2 * HID], bf16, tag="kvh")
            for j in range(JW):
                mm = psT.tile([P, 2 * HID], f32, tag="tr", name="mmB")
                nc.tensor.matmul(
                    mm[:, 0:HID], lhsT=xk[:, ts(j, P)], rhs=c_wk[:], start=True, stop=True
                )
                nc.tensor.matmul(
                    mm[:, HID : 2 * HID], lhsT=xv[:, ts(j, P)], rhs=c_wv[:],
                    start=True, stop=True,
                )
                if j % 2 == 0:
                    nc.scalar.copy(kvh[:, j, :], mm[:])
                else:
                    nc.vector.tensor_copy(kvh[:, j, :], mm[:])
            kv_dst = kvf[ts(t, P * JW), :]
            d_ap = kv_dst.ap
            nc.scalar.dma_start(
                bass.AP(
                    kv_dst.tensor,
                    kv_dst.offset,
                    [[d_ap[0][0], P], [d_ap[0][0] * P, JW], list(d_ap[1])],
                ),
                kvh[:],
            )
        # Q projection last: it does not gate the gathers.
        for t in range(NSH // W):
            xt = pp.tile([P, W], bf16, tag="xk")
            nc.sync.dma_start(xt[:], qT[:, ts(t, W)])
            for j in range(JW):
                mm = psT.tile([P, P], f32, tag="tr", name="mmB")
                nc.tensor.matmul(
                    mm[:], lhsT=xt[:, ts(j, P)], rhs=c_wq[:], start=True, stop=True
                )
                nc.vector.tensor_tensor(
                    out=qsb[:, t * JW + j, :],
                    in0=mm[:],
                    in1=c_bqr[:],
                    op=mybir.AluOpType.add,
                )

        # ------------------------- phase E: blocks -------------------
        # Emission is software-pipelined: block b+1's prologue (index loads,
        # gathers, one-hot builds) is emitted before block b's epilogue, and
        # the Q-expansion matmuls run one group ahead of the DVE/agg chain so
        # the PE queue never drains while DVE computes weights.
        # all index tiles are tiny (512B/partition); load them all upfront
        # so a block's first gather never waits on its index load
        idxbs = []
        for b in range(BPC):
            idxb = blkp.tile([P, SRCW], i16, tag="idxb")
            nc.sync.dma_start(idxb[:], gidx[b])
            idxbs.append(idxb)

        def emit_prologue(b):
            idxb = idxbs[b]

            # gather K|V rows by source id: 1024-index gathers (the SWDGE
            # descriptor ring caps a gather at ~2016 indices), two per
            # 2048-edge compute group, into one kv tile
            kvs = []
            for g2 in range(G2):
                kv = kvp.tile([P, 2 * GSUB, 2 * HID], bf16, tag="kv")
                for h in range(2):
                    g = 2 * g2 + h
                    nc.gpsimd.dma_gather(
                        kv[:, h * GSUB : (h + 1) * GSUB, :],
                        kvf[:],
                        idxb[:, g * (GRP // 16) : (g + 1) * (GRP // 16)],
                        GRP,
                        GRP,
                        2 * HID,
                        queue_num=(b * G + g) % 4,
                    )
                kvs.append(kv)

            # host-precomputed one-hot dst matrices, both layouts:
            #   sel_en[e, j, d] = (dst of edge j*128+e == d)  (agg lhsT)
            #   sel_ne[s, e]    = (dst of edge e == s)        (qd lhsT)
            sel_en = selp.tile([P, NSUB, P], bf16, tag="sel_en")
            nc.sync.dma_start(sel_en[:], selE[b])
            sel_ne = selp.tile([P, cap], bf16, tag="sel_ne")
            nc.sync.dma_start(sel_ne[:], selN[b])
            return kvs, sel_en, sel_ne

        def emit_qd(b, g2, sel_ne):
            qd_ps = psQ.tile([P, GRP2], f32, tag="qd")
            for j in range(GSUB2):
                nc.tensor.matmul(
                    qd_ps[:, ts(j, P)],
                    lhsT=sel_ne[:, (g2 * GSUB2 + j) * P : (g2 * GSUB2 + j + 1) * P],
                    rhs=qsb[:, b, :],
                    start=True,
                    stop=True,
                )
            return qd_ps

        def emit_compute(b, kvs, sel_en, sel_ne, agg_ps):
            qd_next = emit_qd(b, 0, sel_ne)
            for g2 in range(G2):
                first = g2 == 0
                last = g2 == G2 - 1
                kv = kvs[g2]
                qd_ps = qd_next

                # cast the Q expansion to packed bf16 SBUF on ScalarE so every
                # DVE operand below is 2-byte/packed/SBUF (DVE 2x mode)
                qd_sb = ep.tile([P, GRP2], bf16, tag="qd_sb")
                nc.scalar.copy(qd_sb[:], qd_ps[:])
                prod = ep.tile([P, GSUB2, P], bf16, tag="prod")
                nc.vector.tensor_tensor(
                    out=prod[:],
                    in0=qd_sb[:].rearrange("p (j e) -> p j e", j=GSUB2),
                    in1=kv[:, :, 0:HID],
                    op=mybir.AluOpType.mult,
                )
                if not last:
                    qd_next = emit_qd(b, g2 + 1, sel_ne)
                scores = ep.tile([P, GSUB2 * H], bf16, tag="scores")
                with nc.allow_low_precision("bf16 scores; softmax tolerates"):
                    nc.vector.reduce_sum(
                        out=scores[:],
                        in_=prod[:].rearrange("p j (h d) -> p (j h) d", d=D),
                        axis=mybir.AxisListType.X,
                    )
                # combined [V-weighted | exp] tile: one agg matmul per sub-tile
                wvx = ep.tile([P, GSUB2, HID + H], bf16, tag="wvx")
                wx_ap = wvx[:]
                wstep = wx_ap.ap[1][0]          # free stride of sub-tile dim
                nc.scalar.activation(
                    ap3(
                        wx_ap,
                        [list(wx_ap.ap[0]), [wstep, GSUB2], [1, H]],
                        extra_offset=HID,
                    ),
                    scores[:].rearrange("p (j h) -> p j h", j=GSUB2),
                    AF.Exp,
                )
                nc.vector.tensor_tensor(
                    out=wvx[:, :, 0:HID].rearrange("p j (h d) -> p j h d", d=D),
                    in0=kv[:, :, HID : 2 * HID].rearrange("p j (h d) -> p j h d", d=D),
                    in1=ap3(
                        wx_ap,
                        [list(wx_ap.ap[0]), [wstep, GSUB2], [1, H], [0, D]],
                        extra_offset=HID,
                    ),
                    op=mybir.AluOpType.mult,
                )

                for j in range(GSUB2):
                    nc.tensor.matmul(
                        agg_ps[:],
                        lhsT=sel_en[:, g2 * GSUB2 + j, :],
                        rhs=wvx[:, j, :],
                        start=first and j == 0,
                        stop=last and j == GSUB2 - 1,
                    )

        def emit_epilogue(b, agg_ps):
            # out = agg * (1/sum_exp)  (bv/bo folded into c_bo on the host),
            # transposed on PE, then the Wo projection.
            recip = ep.tile([P, H], f32, tag="recip")
            nc.vector.reciprocal(recip[:], agg_ps[:, HID : HID + H])
            outn = ep.tile([P, P], bf16, tag="outn")
            for h in range(H):
                nc.scalar.mul(
                    outn[:, h * D : (h + 1) * D],
                    agg_ps[:, h * D : (h + 1) * D],
                    recip[:, h : h + 1],
                )
            trn = psT.tile([P, P], bf16, tag="tr")
            nc.tensor.transpose(trn[:], outn[:], identb[:])
            outnT = ep.tile([P, P], bf16, tag="outnT")
            nc.scalar.copy(outnT[:], trn[:])
            fin_ps = psT.tile([P, P], f32, tag="tr")
            nc.tensor.matmul(fin_ps[:], lhsT=c_wo[:], rhs=outnT[:], start=True, stop=True)
            fin = ep.tile([P, P], f32, tag="fin")
            nc.scalar.activation(fin[:], fin_ps[:], AF.Identity, bias=c_bo[:, 0:1])
            nc.scalar.dma_start(outT[:, ts(b, P)], fin[:])

        # Epilogues are deferred one block: the PE executes block b-1's
        # (long-ready) transpose + Wo matmul right after block b's agg
        # instead of stalling on block b's normalize chain.  psO bufs=2
        # holds exactly the two live aggregation buffers this needs.
        prologue = emit_prologue(0)
        pending = None
        for b in range(BPC):
            kvs, sel_en, sel_ne = prologue
            agg_ps = psO.tile([P, HID + H], f32, tag="aggp")   # [n, f | h]
            emit_compute(b, kvs, sel_en, sel_ne, agg_ps)
            if b + 1 < BPC:
                prologue = emit_prologue(b + 1)
            if pending is not None:
                emit_epilogue(*pending)
            pending = (b, agg_ps)
        emit_epilogue(*pending)

    nc.compile()
    return nc


# ---------------------------------------------------------------- entrypoint
def kernel(**inputs):
    from concourse import bass_utils

    perm, G, src_pad, dstloc_pad = _build_plan(inputs["edge_index"])
    in_maps = _host_inputs(inputs, perm, G, src_pad, dstloc_pad)

    if G not in _COMPILED:
        _COMPILED[G] = _build_nc(G)
    nc = _COMPILED[G]

    res = bass_utils.run_bass_kernel_spmd(nc, in_maps, core_ids=list(range(NCORES)))
    out_pad = np.concatenate(
        [np.asarray(res.results[c]["outT"]).T for c in range(NCORES)], axis=0
    )
    return np.ascontiguousarray(out_pad[perm])
